# revision 1
# baseline (speedup 1.0000x reference)
"""Trainium2 Bass kernel for the GCM (global context module) problem.

Computation per batch sample b (x_b = x[b] viewed as [C=512, HW=9216]):
    x1 = w1 @ x_b                      [128, HW]
    x2 = w2 @ x_b                      [256, HW]
    v  = softmax_all(x1 @ x2^T)        [128, 256]  (softmax over all 32768)
    n  = relu(v + w3 @ v)              [128, 256]
    z  = w4 @ n^T                      [256, 128]
    W  = w5 @ z                        [512, 128]  (collapses y/conv5: w5@(z@x1) == (w5@z)@x1)
    out = x_b + W @ x1                 [512, HW]

Sharding: data-parallel over batch, one sample per NeuronCore (8 cores).

On-chip strategy per core:
  Phase 1: stream x in ([128,1536] tiles, resident in SBUF), compute
    hw-major [x1T|x2T] tiles via matmul with the X-slice as the stationary
    operand (out[hw,384] = X_slice.T @ [w1T|w2T]), accumulate
    v = x1T.T @ x2T in a persistent PSUM bank over 72 subtiles.
  Softmax: global max/sum via DVE free-dim reduce + GPSIMD partition
    all-reduce; exp on ScalarE with -max bias; normalize by 1/sum.
  Small chain: conv3+relu, PE transposes for n^T, z, W^T = z^T @ w5^T.
  Phase 2: per 512-wide tile recompute x1 (k-major) from resident x,
    x_res = W @ x1 via W^T slices as stationary, residual add on DVE
    (exact f32 read of resident x), DMA out.

All matmuls run as float32r (full PE rate at N>=256) on f32 bits; the
residual add is exact f32 (f32r storage bitcast back to f32 — same bits).
Numerically safe: the softmax here is a near-argmax (gaps >> f32r
rounding) and |x_res| << |x|.
"""

import numpy as np

import concourse.bass as bass
import concourse.tile as tile
from concourse import bacc, mybir, bass_isa
from concourse.bass_utils import run_bass_kernel_spmd
from concourse.masks import make_identity

F32 = mybir.dt.float32
F32R = mybir.dt.float32r
BF16 = mybir.dt.bfloat16
AX = mybir.AxisListType
AL = mybir.AluOpType
AF = mybir.ActivationFunctionType

N_CORES = 8
C = 512
H = W_IMG = 96
HW = H * W_IMG          # 9216
CK = C // 128           # 4 chunks of channels
NBLK = 6                # x blocks along hw
BLK = HW // NBLK        # 1536
NSUB = HW // 128        # 72 phase-1 subtiles
SUB_PER_BLK = BLK // 128
NT = HW // 512          # 18 phase-2 tiles
T_PER_BLK = BLK // 512
C4 = C // 4             # 128
C2 = C // 2             # 256
KM = C4 + C2            # 384 = concat(x1T, x2T) free size


def _emit(ctx, tc, aps, use_bias):
    nc = tc.nc
    x_d = aps["x"]
    w12t_d = aps["w12t"]
    w3t_d = aps["w3t"]
    w4t_d = aps["w4t"]
    w5t_d = aps["w5t"]
    out_d = aps["out"]

    consts = ctx.enter_context(tc.tile_pool(name="consts", bufs=1))

    # Identity comes from HBM (host-provided) instead of gpsimd memset +
    # affine_select: the gpsimd path sits behind the ~15us mlp-library
    # ucode load and delayed the PE warmup to ~17us.
    ident_d = aps["ident"]
    ident = consts.tile([128, 128], F32, tag="ident")
    nc.sync.dma_start(out=ident[:], in_=ident_d[:, :])
    identr = consts.tile([128, 128], F32R, tag="identr")
    nc.sync.dma_start(out=identr[:], in_=ident_d[:, :].bitcast(F32R))

    # ---- weights to SBUF (f32r: consumed only by matmuls) ----
    w12 = []
    for c in range(CK):
        t = consts.tile([128, KM], F32R, tag=f"w12_{c}")
        nc.sync.dma_start(out=t[:], in_=w12t_d[c * 128 : (c + 1) * 128, :])
        w12.append(t)
    w3t = consts.tile([128, 128], F32R, tag="w3t")
    nc.sync.dma_start(out=w3t[:], in_=w3t_d[:, :])
    w4t = []
    for q in range(2):
        t = consts.tile([128, C2], F32R, tag=f"w4t_{q}")
        nc.sync.dma_start(out=t[:], in_=w4t_d[q * 128 : (q + 1) * 128, :])
        w4t.append(t)
    w5t = []
    for q in range(2):
        t = consts.tile([128, C], F32R, tag=f"w5t_{q}")
        nc.sync.dma_start(out=t[:], in_=w5t_d[q * 128 : (q + 1) * 128, :])
        w5t.append(t)

    bias_t = {}
    if use_bias:
        b12row_d = aps["b12row"]
        b1_d, b3_d, b4_d, b5_d = aps["b1c"], aps["b3c"], aps["b4c"], aps["b5c"]
        # [b1|b2] replicated across partitions, added to the hw-major tiles
        brow1 = consts.tile([1, KM], F32, tag="brow1")
        nc.sync.dma_start(out=brow1[:], in_=b12row_d[:, :])
        brow = consts.tile([128, KM], F32, tag="brow")
        nc.gpsimd.partition_broadcast(brow[:], brow1[:])
        bias_t["brow"] = brow
        b1 = consts.tile([128, 1], F32, tag="b1")
        nc.sync.dma_start(out=b1[:], in_=b1_d[:, :])
        bias_t["b1"] = b1
        b3 = consts.tile([128, 1], F32, tag="b3")
        nc.sync.dma_start(out=b3[:], in_=b3_d[:, :])
        bias_t["b3"] = b3
        b4 = []
        for q in range(2):
            t = consts.tile([128, 1], F32, tag=f"b4_{q}")
            nc.sync.dma_start(out=t[:], in_=b4_d[q * 128 : (q + 1) * 128, :])
            b4.append(t)
        bias_t["b4"] = b4
        b5 = []
        for oc in range(CK):
            t = consts.tile([128, 1], F32, tag=f"b5_{oc}")
            nc.sync.dma_start(out=t[:], in_=b5_d[oc * 128 : (oc + 1) * 128, :])
            b5.append(t)
        bias_t["b5"] = b5

    # ---- x resident in SBUF: 24 tiles [128, 1536] f32r ----
    # Block 0 is DMA'd in [128, 512] pieces (c-interleaved) so the first
    # compute subtiles become ready ~3x sooner; later blocks use one big
    # transfer each for bandwidth.
    xpool = ctx.enter_context(tc.tile_pool(name="x", bufs=1))
    xt = {}
    for b in range(NBLK):
        for c in range(CK):
            xt[(c, b)] = xpool.tile(
                [128, BLK], F32R, tag=f"x_{c}_{b}", name=f"x_{c}_{b}"
            )
    for p in range(BLK // 512):
        for c in range(CK):
            nc.sync.dma_start(
                out=xt[(c, 0)][:, p * 512 : (p + 1) * 512],
                in_=x_d[c * 128 : (c + 1) * 128, p * 512 : (p + 1) * 512],
            )
    for b in range(1, NBLK):
        for c in range(CK):
            nc.sync.dma_start(
                out=xt[(c, b)][:],
                in_=x_d[c * 128 : (c + 1) * 128, b * BLK : (b + 1) * BLK],
            )

    sm = ctx.enter_context(tc.tile_pool(name="sm", bufs=1))

    # ---- phase 1: x12T tiles + v accumulation ----
    with (
        tc.tile_pool(name="psA", bufs=3, space="PSUM") as psA,
        tc.tile_pool(name="vps", bufs=1, space="PSUM") as vps,
        tc.tile_pool(name="xtp", bufs=6) as xtp,
    ):
        v_ps = vps.tile([128, C2], F32, tag="v")

        # Warm the PE HAM clock-gate during the initial x-DMA window: ~6us of
        # dummy matmuls (no data deps) so real phase-1 matmuls start at 2.4GHz.
        wps = psA.tile([128, 128], F32, tag="warm", bufs=1)
        for _ in range(30):
            nc.tensor.matmul(wps[:], identr[:], identr[:], start=True, stop=True)

        def emit_v(s, xtile):
            nc.tensor.matmul(
                v_ps[:],
                xtile[:, 0:C4],
                xtile[:, C4:KM],
                start=(s == 0),
                stop=(s == NSUB - 1),
            )

        SKEW = 2
        pend = []
        for s in range(NSUB):
            b, off = divmod(s, SUB_PER_BLK)
            off *= 128
            ps = psA.tile([128, KM], F32, tag="xts")
            for c in range(CK):
                nc.tensor.matmul(
                    ps[:],
                    xt[(c, b)][:, off : off + 128],
                    w12[c][:],
                    start=(c == 0),
                    stop=(c == CK - 1),
                )
            xtile = xtp.tile([128, KM], F32R, tag="xt")
            if use_bias:
                nc.vector.tensor_tensor(
                    xtile[:], ps[:], bias_t["brow"][:], op=AL.add
                )
            else:
                nc.scalar.copy(xtile[:], ps[:])
            pend.append((s, xtile))
            if len(pend) > SKEW:
                emit_v(*pend.pop(0))
        while pend:
            emit_v(*pend.pop(0))

        # ---- softmax over all 32768 entries of v ----
        m1 = sm.tile([128, 1], F32, tag="m1")
        nc.vector.tensor_reduce(m1[:], v_ps[:], axis=AX.X, op=AL.max)
        mall = sm.tile([128, 1], F32, tag="mall")
        nc.gpsimd.partition_all_reduce(mall[:], m1[:], 128, bass_isa.ReduceOp.max)
        negm = sm.tile([128, 1], F32, tag="negm")
        nc.vector.tensor_scalar_mul(negm[:], mall[:], -1.0)
        e = sm.tile([128, C2], F32, tag="e")
        nc.scalar.activation(e[:], v_ps[:], AF.Exp, bias=negm[:], scale=1.0)

    s1 = sm.tile([128, 1], F32, tag="s1")
    nc.vector.tensor_reduce(s1[:], e[:], axis=AX.X, op=AL.add)
    sall = sm.tile([128, 1], F32, tag="sall")
    nc.gpsimd.partition_all_reduce(sall[:], s1[:], 128, bass_isa.ReduceOp.add)
    sinv = sm.tile([128, 1], F32, tag="sinv")
    nc.vector.reciprocal(sinv[:], sall[:])
    en = sm.tile([128, C2], F32R, tag="en")
    nc.vector.tensor_scalar_mul(en[:], e[:], sinv[:])

    # ---- small chain: conv3+relu, n^T, z, W^T ----
    wt = sm.tile([128, C], BF16, tag="wt")
    with tc.tile_pool(name="psB", bufs=2, space="PSUM") as psB:
        ps3 = psB.tile([128, C2], F32, tag="ps3")
        nc.tensor.matmul(ps3[:], w3t[:], en[:], start=True, stop=True)
        nsb = sm.tile([128, C2], F32, tag="nsb")
        b3s = bias_t["b3"][:] if use_bias else 0.0
        nc.vector.scalar_tensor_tensor(
            nsb[:], ps3[:], b3s, en[:].bitcast(F32), op0=AL.add, op1=AL.add
        )
        nc.vector.tensor_scalar_max(nsb[:], nsb[:], 0.0)

        nts = []
        for q in range(2):
            pT = psB.tile([128, 128], F32, tag="pT")
            nc.tensor.transpose(pT[:], nsb[:, q * 128 : (q + 1) * 128], ident[:])
            ntq = sm.tile([128, 128], F32R, tag=f"nt{q}")
            nc.scalar.copy(ntq[:], pT[:])
            nts.append(ntq)

        zs = []
        for mc in range(2):
            pz = psB.tile([128, 128], F32, tag="pz")
            for q in range(2):
                nc.tensor.matmul(
                    pz[:],
                    w4t[q][:, mc * 128 : (mc + 1) * 128],
                    nts[q][:],
                    start=(q == 0),
                    stop=(q == 1),
                )
            zq = sm.tile([128, 128], F32R, tag=f"z{mc}")
            if use_bias:
                nc.scalar.add(zq[:], pz[:], bias_t["b4"][mc][:])
            else:
                nc.scalar.copy(zq[:], pz[:])
            zs.append(zq)

        pW = psB.tile([128, C], F32, tag="pW")
        for mc in range(2):
            nc.tensor.matmul(
                pW[:], zs[mc][:], w5t[mc][:], start=(mc == 0), stop=(mc == 1)
            )
        nc.scalar.copy(wt[:], pW[:])

    # ---- phase 2: x1 recompute (bf16), x_res = W @ x1, residual, DMA out ----
    # Processed in groups of 3 hw-tiles (1536 cols): x1 for the group, then
    # per output-channel chunk the 3 x_res matmuls + residual adds land in
    # one [128, 1536] staging tile, DMA'd as a single 768 KB transfer on
    # alternating HWDGE engines (sync/scalar) to keep the DMA rings fed.
    # Residual alternates between the PE (f32r identity matmul into the
    # x_res PSUM bank; f32r keeps 12 mantissa bits -> ~2.4e-4 worst-case on
    # the passthrough) and a DVE tensor_tensor add (exact f32), balancing
    # PE / DVE / ACT under the output-DMA floor.
    with (
        tc.tile_pool(name="psC", bufs=2, space="PSUM") as psC,
        tc.tile_pool(name="psD", bufs=5, space="PSUM") as psD,
        tc.tile_pool(name="x1p", bufs=4) as x1p,
        tc.tile_pool(name="outp", bufs=8) as outp,
    ):

        def emit_tail(t, x1tile):
            b, off = divmod(t, T_PER_BLK)
            off *= 512
            goff = t * 512
            for oc in range(CK):
                pr = psD.tile([128, 512], F32, tag="pr")
                nc.tensor.matmul(
                    pr[:],
                    wt[:, oc * 128 : (oc + 1) * 128],
                    x1tile[:],
                    start=True,
                    stop=False,
                )
                nc.tensor.matmul(
                    pr[:],
                    identr[:],
                    xt[(oc, b)][:, off : off + 512],
                    start=False,
                    stop=True,
                )
                ot = outp.tile([128, 512], F32, tag="ot")
                b5s = bias_t["b5"][oc][:] if use_bias else None
                if (t * CK + oc) % 2 == 0:
                    if b5s is not None:
                        nc.scalar.add(ot[:], pr[:], b5s)
                    else:
                        nc.scalar.copy(ot[:], pr[:])
                else:
                    if b5s is not None:
                        nc.vector.tensor_scalar_add(ot[:], pr[:], b5s)
                    else:
                        nc.vector.tensor_copy(ot[:], pr[:])
                nc.sync.dma_start(
                    out=out_d[oc * 128 : (oc + 1) * 128, goff : goff + 512],
                    in_=ot[:],
                )

        prev = None
        for t in range(NT):
            b, off = divmod(t, T_PER_BLK)
            off *= 512
            px1 = psC.tile([128, 512], F32, tag="px1")
            for c in range(CK):
                nc.tensor.matmul(
                    px1[:],
                    w12[c][:, 0:C4],
                    xt[(c, b)][:, off : off + 512],
                    start=(c == 0),
                    stop=(c == CK - 1),
                )
            x1tile = x1p.tile([128, 512], BF16, tag="x1t")
            if use_bias:
                nc.scalar.add(x1tile[:], px1[:], bias_t["b1"][:])
            else:
                nc.scalar.copy(x1tile[:], px1[:])
            if prev is not None:
                emit_tail(*prev)
            prev = (t, x1tile)
        emit_tail(*prev)


def _build(use_bias):
    nc = bacc.Bacc("TRN2", target_bir_lowering=False, debug=False, num_devices=N_CORES)
    aps = {
        "x": nc.dram_tensor("x", [C, HW], F32R, kind="ExternalInput").ap(),
        "w12t": nc.dram_tensor("w12t", [C, KM], F32R, kind="ExternalInput").ap(),
        "w3t": nc.dram_tensor("w3t", [C4, C4], F32R, kind="ExternalInput").ap(),
        "w4t": nc.dram_tensor("w4t", [C2, C2], F32R, kind="ExternalInput").ap(),
        "w5t": nc.dram_tensor("w5t", [C2, C], F32R, kind="ExternalInput").ap(),
        "ident": nc.dram_tensor("ident", [128, 128], F32, kind="ExternalInput").ap(),
        "out": nc.dram_tensor("out", [C, HW], F32, kind="ExternalOutput").ap(),
    }
    if use_bias:
        aps["b12row"] = nc.dram_tensor(
            "b12row", [1, KM], F32, kind="ExternalInput"
        ).ap()
        aps["b1c"] = nc.dram_tensor("b1c", [C4, 1], F32, kind="ExternalInput").ap()
        aps["b3c"] = nc.dram_tensor("b3c", [C4, 1], F32, kind="ExternalInput").ap()
        aps["b4c"] = nc.dram_tensor("b4c", [C2, 1], F32, kind="ExternalInput").ap()
        aps["b5c"] = nc.dram_tensor("b5c", [C, 1], F32, kind="ExternalInput").ap()

    from contextlib import ExitStack

    with tile.TileContext(nc) as tc:
        with ExitStack() as ctx:
            _emit(ctx, tc, aps, use_bias)
    nc.compile()
    return nc


_CACHE = {}


def _run(inputs, trace=False, **run_kwargs):
    x = np.ascontiguousarray(np.asarray(inputs["x"], dtype=np.float32))
    assert x.shape == (N_CORES, C, H, W_IMG), x.shape
    w1 = np.asarray(inputs["w1"], dtype=np.float32)
    w2 = np.asarray(inputs["w2"], dtype=np.float32)
    w3 = np.asarray(inputs["w3"], dtype=np.float32)
    w4 = np.asarray(inputs["w4"], dtype=np.float32)
    w5 = np.asarray(inputs["w5"], dtype=np.float32)
    b1 = np.asarray(inputs["b1"], dtype=np.float32)
    b2 = np.asarray(inputs["b2"], dtype=np.float32)
    b3 = np.asarray(inputs["b3"], dtype=np.float32)
    b4 = np.asarray(inputs["b4"], dtype=np.float32)
    b5 = np.asarray(inputs["b5"], dtype=np.float32)
    use_bias = bool(
        np.any(b1) or np.any(b2) or np.any(b3) or np.any(b4) or np.any(b5)
    )

    if use_bias not in _CACHE:
        _CACHE[use_bias] = _build(use_bias)
    nc = _CACHE[use_bias]

    w12t = np.ascontiguousarray(
        np.concatenate([w1.T, w2.T], axis=1), dtype=np.float32
    )  # [512, 384]
    w3t = np.ascontiguousarray(w3.T)
    w4t = np.ascontiguousarray(w4.T)
    w5t = np.ascontiguousarray(w5.T)

    shared = {
        "w12t": w12t,
        "w3t": w3t,
        "w4t": w4t,
        "w5t": w5t,
        "ident": np.eye(128, dtype=np.float32),
    }
    if use_bias:
        shared["b12row"] = np.ascontiguousarray(
            np.concatenate([b1, b2])[None, :], dtype=np.float32
        )
        shared["b1c"] = np.ascontiguousarray(b1[:, None])
        shared["b3c"] = np.ascontiguousarray(b3[:, None])
        shared["b4c"] = np.ascontiguousarray(b4[:, None])
        shared["b5c"] = np.ascontiguousarray(b5[:, None])

    in_maps = [
        {"x": np.ascontiguousarray(x[b].reshape(C, HW)), **shared}
        for b in range(N_CORES)
    ]
    res = run_bass_kernel_spmd(
        nc, in_maps, core_ids=list(range(N_CORES)), trace=trace, **run_kwargs
    )
    out = np.stack(
        [res.results[b]["out"].reshape(C, H, W_IMG) for b in range(N_CORES)]
    ).astype(np.float32)
    return out, res


def kernel(**inputs):
    out, _ = _run(inputs, trace=False)
    return out



# revision 6
# speedup vs baseline: 1.2694x; 1.2694x over previous
"""Trainium2 Bass kernel for the GCM (global context module) problem.

Computation per batch sample b (x_b = x[b] viewed as [C=512, HW=9216]):
    x1 = w1 @ x_b                      [128, HW]
    x2 = w2 @ x_b                      [256, HW]
    v  = softmax_all(x1 @ x2^T)        [128, 256]  (softmax over all 32768)
    n  = relu(v + w3 @ v)              [128, 256]
    z  = w4 @ n^T                      [256, 128]
    W  = w5 @ z                        [512, 128]  (collapses y/conv5: w5@(z@x1) == (w5@z)@x1)
    out = x_b + W @ x1                 [512, HW]

Sharding: data-parallel over batch, one sample per NeuronCore (8 cores).

v2 strategy (vs f32r v1): everything bf16.
  - x is uploaded as bf16 (9.4 MB vs 18.9), out is written bf16 and
    upconverted on host.  Measured end-to-end rel err ~5e-3 vs the 2e-2
    gate (the softmax is a near-one-hot argmax with top-2 logit gaps of
    6.7..102, and |x_res| <= 0.04 vs |x| <= 5.1, so reduced precision in
    the v-chain is harmless; the bf16 rounding of the x passthrough and
    of out dominates the error).
  - All matmuls bf16 (1 cycle/row at any N, same PE rate as f32r@N>=256)
    with f32 PSUM accumulation.
  - Mid-kernel HAM keep-warm: dummy identity matmuls bridge the serial
    softmax window so phase 2 starts at 2.4 GHz instead of 1.2.
  - Out staged in [128,1536] bf16 tiles, 24 DMAs alternating sync/scalar
    HWDGE queues.
"""

import numpy as np
import ml_dtypes

import concourse.bass as bass
import concourse.tile as tile
from concourse import bacc, mybir, bass_isa
from concourse.bass_utils import run_bass_kernel_spmd

F32 = mybir.dt.float32
BF16 = mybir.dt.bfloat16
AX = mybir.AxisListType
AL = mybir.AluOpType
AF = mybir.ActivationFunctionType

N_CORES = 8
C = 512
H = W_IMG = 96
HW = H * W_IMG          # 9216
CK = C // 128           # 4 chunks of channels
NBLK = 6                # x blocks along hw
BLK = HW // NBLK        # 1536
NSUB = HW // 128        # 72 phase-1 subtiles
SUB_PER_BLK = BLK // 128
NT = HW // 512          # 18 phase-2 tiles
T_PER_BLK = BLK // 512
C4 = C // 4             # 128
C2 = C // 2             # 256
KM = C4 + C2            # 384 = concat(x1T, x2T) free size


def _emit(ctx, tc, aps, use_bias):
    nc = tc.nc
    x_d = aps["x"]
    w12t_d = aps["w12t"]
    w3t_d = aps["w3t"]
    w4t_d = aps["w4t"]
    w5t_d = aps["w5t"]
    out_d = aps["out"]

    consts = ctx.enter_context(tc.tile_pool(name="consts", bufs=1))

    # identities from HBM (f32 for the small f32 chain transposes, bf16 for
    # warmup + residual passthrough matmuls)
    identf = consts.tile([128, 128], F32, tag="identf")
    nc.sync.dma_start(out=identf[:], in_=aps["identf"][:, :])
    identb = consts.tile([128, 128], BF16, tag="identb")
    nc.sync.dma_start(out=identb[:], in_=aps["identb"][:, :])

    # ---- x resident in SBUF: 24 tiles [128, 1536] bf16 ----
    # Block 0 is DMA'd in [128, 512] pieces (c-interleaved) so the first
    # compute subtiles become ready sooner; later blocks use one transfer.
    xpool = ctx.enter_context(tc.tile_pool(name="x", bufs=1))
    xt = {}
    for b in range(NBLK):
        for c in range(CK):
            xt[(c, b)] = xpool.tile(
                [128, BLK], BF16, tag=f"x_{c}_{b}", name=f"x_{c}_{b}"
            )
    for p in range(BLK // 512):
        for c in range(CK):
            nc.sync.dma_start(
                out=xt[(c, 0)][:, p * 512 : (p + 1) * 512],
                in_=x_d[c * 128 : (c + 1) * 128, p * 512 : (p + 1) * 512],
            )

    # ---- weights to SBUF (bf16), issued on the scalar HWDGE queue so they
    # stream in parallel with the x block DMAs on sync ----
    w12 = []
    for c in range(CK):
        t = consts.tile([128, KM], BF16, tag=f"w12_{c}")
        nc.scalar.dma_start(out=t[:], in_=w12t_d[c * 128 : (c + 1) * 128, :])
        w12.append(t)
    w3t = consts.tile([128, 128], BF16, tag="w3t")
    nc.scalar.dma_start(out=w3t[:], in_=w3t_d[:, :])
    w4t = []
    for q in range(2):
        t = consts.tile([128, C2], BF16, tag=f"w4t_{q}")
        nc.scalar.dma_start(out=t[:], in_=w4t_d[q * 128 : (q + 1) * 128, :])
        w4t.append(t)
    w5t = []
    for q in range(2):
        t = consts.tile([128, C], BF16, tag=f"w5t_{q}")
        nc.scalar.dma_start(out=t[:], in_=w5t_d[q * 128 : (q + 1) * 128, :])
        w5t.append(t)

    bias_t = {}
    if use_bias:
        b12row_d = aps["b12row"]
        b1_d, b3_d, b4_d, b5_d = aps["b1c"], aps["b3c"], aps["b4c"], aps["b5c"]
        brow1 = consts.tile([1, KM], F32, tag="brow1")
        nc.scalar.dma_start(out=brow1[:], in_=b12row_d[:, :])
        brow = consts.tile([128, KM], F32, tag="brow")
        nc.gpsimd.partition_broadcast(brow[:], brow1[:])
        bias_t["brow"] = brow
        b1 = consts.tile([128, 1], F32, tag="b1")
        nc.scalar.dma_start(out=b1[:], in_=b1_d[:, :])
        bias_t["b1"] = b1
        b3 = consts.tile([128, 1], F32, tag="b3")
        nc.scalar.dma_start(out=b3[:], in_=b3_d[:, :])
        bias_t["b3"] = b3
        b4 = []
        for q in range(2):
            t = consts.tile([128, 1], F32, tag=f"b4_{q}")
            nc.scalar.dma_start(out=t[:], in_=b4_d[q * 128 : (q + 1) * 128, :])
            b4.append(t)
        bias_t["b4"] = b4
        b5 = []
        for oc in range(CK):
            t = consts.tile([128, 1], F32, tag=f"b5_{oc}")
            nc.scalar.dma_start(out=t[:], in_=b5_d[oc * 128 : (oc + 1) * 128, :])
            b5.append(t)
        bias_t["b5"] = b5

    # remaining x blocks
    for b in range(1, NBLK):
        for c in range(CK):
            nc.sync.dma_start(
                out=xt[(c, b)][:],
                in_=x_d[c * 128 : (c + 1) * 128, b * BLK : (b + 1) * BLK],
            )

    sm = ctx.enter_context(tc.tile_pool(name="sm", bufs=1))

    # ---- phase 1: x12T tiles + v accumulation ----
    with (
        tc.tile_pool(name="psA", bufs=3, space="PSUM") as psA,
        tc.tile_pool(name="vps", bufs=1, space="PSUM") as vps,
        tc.tile_pool(name="xtp", bufs=6) as xtp,
    ):
        v_ps = vps.tile([128, C2], F32, tag="v")

        # Warm the PE HAM clock-gate during the initial x-DMA window.
        wps = psA.tile([128, 128], F32, tag="warm", bufs=1)
        for _ in range(30):
            nc.tensor.matmul(wps[:], identb[:], identb[:], start=True, stop=True)

        def emit_v(s, xtile):
            nc.tensor.matmul(
                v_ps[:],
                xtile[:, 0:C4],
                xtile[:, C4:KM],
                start=(s == 0),
                stop=(s == NSUB - 1),
            )

        SKEW = 2
        pend = []
        for s in range(NSUB):
            b, off = divmod(s, SUB_PER_BLK)
            off *= 128
            ps = psA.tile([128, KM], F32, tag="xts")
            for c in range(CK):
                nc.tensor.matmul(
                    ps[:],
                    xt[(c, b)][:, off : off + 128],
                    w12[c][:],
                    start=(c == 0),
                    stop=(c == CK - 1),
                )
            xtile = xtp.tile([128, KM], BF16, tag="xt")
            if use_bias:
                nc.vector.tensor_tensor(
                    xtile[:], ps[:], bias_t["brow"][:], op=AL.add
                )
            else:
                nc.scalar.copy(xtile[:], ps[:])
            pend.append((s, xtile))
            if len(pend) > SKEW:
                emit_v(*pend.pop(0))
        while pend:
            emit_v(*pend.pop(0))

        # ---- softmax over all 32768 entries of v ----
        m1 = sm.tile([128, 1], F32, tag="m1")
        nc.vector.tensor_reduce(m1[:], v_ps[:], axis=AX.X, op=AL.max)
        mall = sm.tile([128, 1], F32, tag="mall")
        nc.gpsimd.partition_all_reduce(mall[:], m1[:], 128, bass_isa.ReduceOp.max)
        negm = sm.tile([128, 1], F32, tag="negm")
        nc.vector.tensor_scalar_mul(negm[:], mall[:], -1.0)
        e = sm.tile([128, C2], F32, tag="e")
        nc.scalar.activation(e[:], v_ps[:], AF.Exp, bias=negm[:], scale=1.0)

    s1 = sm.tile([128, 1], F32, tag="s1")
    nc.vector.tensor_reduce(s1[:], e[:], axis=AX.X, op=AL.add)
    sall = sm.tile([128, 1], F32, tag="sall")
    nc.gpsimd.partition_all_reduce(sall[:], s1[:], 128, bass_isa.ReduceOp.add)
    sinv = sm.tile([128, 1], F32, tag="sinv")
    nc.vector.reciprocal(sinv[:], sall[:])
    en = sm.tile([128, C2], BF16, tag="en")
    nc.vector.tensor_scalar_mul(en[:], e[:], sinv[:])

    # ---- small chain: conv3+relu, n^T, z, W^T ----
    wt = sm.tile([128, C], BF16, tag="wt")
    with tc.tile_pool(name="psB", bufs=2, space="PSUM") as psB:
        # keep-warm dummies: bridge the serial softmax window so the PE HAM
        # clock does not re-throttle before phase 2.  Shares the pT tag's
        # PSUM ring (the real transposes come later and just rotate it).
        wps2 = psB.tile([128, 128], F32, tag="pT", name="warm2")
        for _ in range(56):
            nc.tensor.matmul(wps2[:], identb[:], identb[:], start=True, stop=True)

        ps3 = psB.tile([128, C2], F32, tag="ps3")
        nc.tensor.matmul(ps3[:], w3t[:], en[:], start=True, stop=True)
        nsb = sm.tile([128, C2], F32, tag="nsb")
        # nsb = (e * sinv) + ps3 (+ b3)
        nc.vector.scalar_tensor_tensor(
            nsb[:], e[:], sinv[:], ps3[:], op0=AL.mult, op1=AL.add
        )
        if use_bias:
            nc.vector.tensor_scalar_add(nsb[:], nsb[:], bias_t["b3"][:])
        nc.vector.tensor_scalar_max(nsb[:], nsb[:], 0.0)

        nts = []
        for q in range(2):
            pT = psB.tile([128, 128], F32, tag="pT")
            nc.tensor.transpose(pT[:], nsb[:, q * 128 : (q + 1) * 128], identf[:])
            ntq = sm.tile([128, 128], BF16, tag=f"nt{q}")
            nc.scalar.copy(ntq[:], pT[:])
            nts.append(ntq)

        zs = []
        for mc in range(2):
            pz = psB.tile([128, 128], F32, tag="pz")
            for q in range(2):
                nc.tensor.matmul(
                    pz[:],
                    w4t[q][:, mc * 128 : (mc + 1) * 128],
                    nts[q][:],
                    start=(q == 0),
                    stop=(q == 1),
                )
            zq = sm.tile([128, 128], BF16, tag=f"z{mc}")
            if use_bias:
                nc.scalar.add(zq[:], pz[:], bias_t["b4"][mc][:])
            else:
                nc.scalar.copy(zq[:], pz[:])
            zs.append(zq)

        pW = psB.tile([128, C], F32, tag="pW")
        for mc in range(2):
            nc.tensor.matmul(
                pW[:], zs[mc][:], w5t[mc][:], start=(mc == 0), stop=(mc == 1)
            )
        nc.scalar.copy(wt[:], pW[:])

    # ---- phase 2: x1 recompute (bf16), x_res = W @ x1, residual, DMA out ----
    # Output is staged in [128, 1536] bf16 tiles (one per (oc, group-of-3
    # hw-tiles)); 24 DMAs alternate between the sync and scalar HWDGE
    # queues.  The residual add alternates between the PE (identity matmul
    # accumulated into the x_res PSUM bank, then an ACT copy to staging)
    # and a DVE tensor_tensor add that writes staging directly.
    with (
        tc.tile_pool(name="psC", bufs=2, space="PSUM") as psC,
        tc.tile_pool(name="psD", bufs=5, space="PSUM") as psD,
        tc.tile_pool(name="x1p", bufs=4) as x1p,
        tc.tile_pool(name="outp", bufs=8) as outp,
    ):
        stage = {}  # oc -> current [128,1536] staging tile

        def emit_tail(t, x1tile):
            b, off = divmod(t, T_PER_BLK)
            off *= 512
            g, gi = divmod(t, 3)
            goff = g * 1536
            for oc in range(CK):
                if gi == 0:
                    stage[oc] = outp.tile(
                        [128, 1536], BF16, tag=f"st{oc}", name=f"st{oc}_{g}"
                    )
                st = stage[oc]
                dst = st[:, gi * 512 : (gi + 1) * 512]
                pr = psD.tile([128, 512], F32, tag="pr")
                use_pe = (t * CK + oc) % 2 == 0
                nc.tensor.matmul(
                    pr[:],
                    wt[:, oc * 128 : (oc + 1) * 128],
                    x1tile[:],
                    start=True,
                    stop=not use_pe,
                )
                b5s = bias_t["b5"][oc][:] if use_bias else None
                if use_pe:
                    nc.tensor.matmul(
                        pr[:],
                        identb[:],
                        xt[(oc, b)][:, off : off + 512],
                        start=False,
                        stop=True,
                    )
                    if b5s is not None:
                        nc.scalar.add(dst, pr[:], b5s)
                    else:
                        nc.scalar.copy(dst, pr[:])
                else:
                    if b5s is not None:
                        nc.vector.scalar_tensor_tensor(
                            dst, pr[:], b5s, xt[(oc, b)][:, off : off + 512],
                            op0=AL.add, op1=AL.add,
                        )
                    else:
                        nc.vector.tensor_tensor(
                            dst, pr[:], xt[(oc, b)][:, off : off + 512], op=AL.add
                        )
                if gi == 2:
                    eng = nc.sync if (oc + g) % 2 == 0 else nc.scalar
                    eng.dma_start(
                        out=out_d[oc * 128 : (oc + 1) * 128, goff : goff + 1536],
                        in_=st[:],
                    )

        prev = None
        for t in range(NT):
            b, off = divmod(t, T_PER_BLK)
            off *= 512
            px1 = psC.tile([128, 512], F32, tag="px1")
            for c in range(CK):
                nc.tensor.matmul(
                    px1[:],
                    w12[c][:, 0:C4],
                    xt[(c, b)][:, off : off + 512],
                    start=(c == 0),
                    stop=(c == CK - 1),
                )
            x1tile = x1p.tile([128, 512], BF16, tag="x1t")
            if use_bias:
                nc.scalar.add(x1tile[:], px1[:], bias_t["b1"][:])
            else:
                nc.scalar.copy(x1tile[:], px1[:])
            if prev is not None:
                emit_tail(*prev)
            prev = (t, x1tile)
        emit_tail(*prev)


def _build(use_bias):
    nc = bacc.Bacc("TRN2", target_bir_lowering=False, debug=False, num_devices=N_CORES)
    aps = {
        "x": nc.dram_tensor("x", [C, HW], BF16, kind="ExternalInput").ap(),
        "w12t": nc.dram_tensor("w12t", [C, KM], BF16, kind="ExternalInput").ap(),
        "w3t": nc.dram_tensor("w3t", [C4, C4], BF16, kind="ExternalInput").ap(),
        "w4t": nc.dram_tensor("w4t", [C2, C2], BF16, kind="ExternalInput").ap(),
        "w5t": nc.dram_tensor("w5t", [C2, C], BF16, kind="ExternalInput").ap(),
        "identf": nc.dram_tensor("identf", [128, 128], F32, kind="ExternalInput").ap(),
        "identb": nc.dram_tensor("identb", [128, 128], BF16, kind="ExternalInput").ap(),
        "out": nc.dram_tensor("out", [C, HW], BF16, kind="ExternalOutput").ap(),
    }
    if use_bias:
        aps["b12row"] = nc.dram_tensor(
            "b12row", [1, KM], F32, kind="ExternalInput"
        ).ap()
        aps["b1c"] = nc.dram_tensor("b1c", [C4, 1], F32, kind="ExternalInput").ap()
        aps["b3c"] = nc.dram_tensor("b3c", [C4, 1], F32, kind="ExternalInput").ap()
        aps["b4c"] = nc.dram_tensor("b4c", [C2, 1], F32, kind="ExternalInput").ap()
        aps["b5c"] = nc.dram_tensor("b5c", [C, 1], F32, kind="ExternalInput").ap()

    from contextlib import ExitStack

    with tile.TileContext(nc) as tc:
        with ExitStack() as ctx:
            _emit(ctx, tc, aps, use_bias)
    nc.compile()
    return nc


_CACHE = {}


def _run(inputs, trace=False, **run_kwargs):
    x = np.ascontiguousarray(np.asarray(inputs["x"], dtype=np.float32))
    assert x.shape == (N_CORES, C, H, W_IMG), x.shape
    w1 = np.asarray(inputs["w1"], dtype=np.float32)
    w2 = np.asarray(inputs["w2"], dtype=np.float32)
    w3 = np.asarray(inputs["w3"], dtype=np.float32)
    w4 = np.asarray(inputs["w4"], dtype=np.float32)
    w5 = np.asarray(inputs["w5"], dtype=np.float32)
    b1 = np.asarray(inputs["b1"], dtype=np.float32)
    b2 = np.asarray(inputs["b2"], dtype=np.float32)
    b3 = np.asarray(inputs["b3"], dtype=np.float32)
    b4 = np.asarray(inputs["b4"], dtype=np.float32)
    b5 = np.asarray(inputs["b5"], dtype=np.float32)
    use_bias = bool(
        np.any(b1) or np.any(b2) or np.any(b3) or np.any(b4) or np.any(b5)
    )

    if use_bias not in _CACHE:
        _CACHE[use_bias] = _build(use_bias)
    nc = _CACHE[use_bias]

    bf = ml_dtypes.bfloat16
    w12t = np.ascontiguousarray(
        np.concatenate([w1.T, w2.T], axis=1), dtype=bf
    )  # [512, 384]
    w3t = np.ascontiguousarray(w3.T, dtype=bf)
    w4t = np.ascontiguousarray(w4.T, dtype=bf)
    w5t = np.ascontiguousarray(w5.T, dtype=bf)

    shared = {
        "w12t": w12t,
        "w3t": w3t,
        "w4t": w4t,
        "w5t": w5t,
        "identf": np.eye(128, dtype=np.float32),
        "identb": np.eye(128, dtype=bf),
    }
    if use_bias:
        shared["b12row"] = np.ascontiguousarray(
            np.concatenate([b1, b2])[None, :], dtype=np.float32
        )
        shared["b1c"] = np.ascontiguousarray(b1[:, None])
        shared["b3c"] = np.ascontiguousarray(b3[:, None])
        shared["b4c"] = np.ascontiguousarray(b4[:, None])
        shared["b5c"] = np.ascontiguousarray(b5[:, None])

    in_maps = [
        {"x": np.ascontiguousarray(x[b].reshape(C, HW).astype(bf)), **shared}
        for b in range(N_CORES)
    ]
    res = run_bass_kernel_spmd(
        nc, in_maps, core_ids=list(range(N_CORES)), trace=trace, **run_kwargs
    )
    out = np.stack(
        [
            np.asarray(res.results[b]["out"]).astype(np.float32).reshape(C, H, W_IMG)
            for b in range(N_CORES)
        ]
    )
    return out, res


def kernel(**inputs):
    out, _ = _run(inputs, trace=False)
    return out


# revision 7
# speedup vs baseline: 1.3162x; 1.0369x over previous
"""Trainium2 Bass kernel for the GCM (global context module) problem.

Computation per batch sample b (x_b = x[b] viewed as [C=512, HW=9216]):
    x1 = w1 @ x_b                      [128, HW]
    x2 = w2 @ x_b                      [256, HW]
    v  = softmax_all(x1 @ x2^T)        [128, 256]  (softmax over all 32768)
    n  = relu(v + w3 @ v)              [128, 256]
    z  = w4 @ n^T                      [256, 128]
    W  = w5 @ z                        [512, 128]  (collapses y/conv5: w5@(z@x1) == (w5@z)@x1)
    out = x_b + W @ x1                 [512, HW]

Sharding: data-parallel over batch, one sample per NeuronCore (8 cores).

v3 strategy: everything bf16, single repacked x upload, DMA off the ACT queue.
  - x uploaded bf16 in a host-repacked layout (one [128, 36864] SBUF tile,
    block-major with a finer-grained block 0) so the whole input streams in
    8 large line-rate DMAs instead of 32 small ones.
  - out written bf16 and upconverted on host.  Measured end-to-end rel err
    ~5e-3 vs the 2e-2 gate (softmax is a near-one-hot argmax with top-2
    logit gaps 6.7..102 and |x_res| <= 0.04 vs |x| <= 5.1; the bf16
    rounding of the x passthrough and of out dominates the error).
  - All matmuls bf16 (1 cycle/row) with f32 PSUM accumulation.
  - PE warmup runs against a memset tile (no DMA dependency) so the HAM
    clock-gate releases before real work arrives; dummy identity matmuls
    bridge the serial softmax window.
  - Out staged in [128,1536] bf16 tiles; all out-DMAs issue on the sync
    HWDGE queue (the scalar queue's ACT copies would delay them).
"""

import numpy as np
import ml_dtypes

import concourse.bass as bass
import concourse.tile as tile
from concourse import bacc, mybir, bass_isa
from concourse.bass_utils import run_bass_kernel_spmd

F32 = mybir.dt.float32
BF16 = mybir.dt.bfloat16
AX = mybir.AxisListType
AL = mybir.AluOpType
AF = mybir.ActivationFunctionType

N_CORES = 8
C = 512
H = W_IMG = 96
HW = H * W_IMG          # 9216
CK = C // 128           # 4 chunks of channels
NBLK = 6                # x blocks along hw
BLK = HW // NBLK        # 1536
NSUB = HW // 128        # 72 phase-1 subtiles
SUB_PER_BLK = BLK // 128
NT = HW // 512          # 18 phase-2 tiles
T_PER_BLK = BLK // 512
C4 = C // 4             # 128
C2 = C // 2             # 256
KM = C4 + C2            # 384 = concat(x1T, x2T) free size
XCOLS = CK * HW         # free size of the packed x tile (36864)


def _xcol(c, goff):
    """Column in the packed x tile for channel-chunk c, global hw offset."""
    b, off = divmod(goff, BLK)
    if b == 0:
        sub, o = divmod(off, 512)
        return sub * (CK * 512) + c * 512 + o
    return b * (CK * BLK) + c * BLK + off


def _emit(ctx, tc, aps, use_bias):
    nc = tc.nc
    x_d = aps["x"]
    w12t_d = aps["w12t"]
    w3t_d = aps["w3t"]
    w4t_d = aps["w4t"]
    w5t_d = aps["w5t"]
    out_d = aps["out"]

    consts = ctx.enter_context(tc.tile_pool(name="consts", bufs=1))

    # Warmup operand: memset, so the first PE matmuls have no DMA dependency.
    warm0 = consts.tile([128, 128], BF16, tag="warm0")
    nc.vector.memset(warm0[:], 0)

    # ---- x resident in SBUF: one packed [128, 36864] bf16 tile ----
    # Host layout: block 0 as 3 sub-blocks of (c0..c3 x 512 cols) for early
    # availability, blocks 1..5 as (c0..c3 x 1536 cols).  8 DMAs on sync.
    xpool = ctx.enter_context(tc.tile_pool(name="x", bufs=1))
    xall = xpool.tile([128, XCOLS], BF16, tag="xall", name="xall")
    for sub in range(3):
        w = CK * 512
        nc.sync.dma_start(
            out=xall[:, sub * w : (sub + 1) * w],
            in_=x_d[:, sub * w : (sub + 1) * w],
        )
    for b in range(1, NBLK):
        w = CK * BLK
        nc.sync.dma_start(
            out=xall[:, b * w : (b + 1) * w],
            in_=x_d[:, b * w : (b + 1) * w],
        )

    def xs(c, goff, width):
        col = _xcol(c, goff)
        return xall[:, col : col + width]

    # ---- weights to SBUF (bf16) on the scalar HWDGE queue (parallel) ----
    w12 = []
    for c in range(CK):
        t = consts.tile([128, KM], BF16, tag=f"w12_{c}")
        nc.scalar.dma_start(out=t[:], in_=w12t_d[c * 128 : (c + 1) * 128, :])
        w12.append(t)
    identb = consts.tile([128, 128], BF16, tag="identb")
    nc.scalar.dma_start(out=identb[:], in_=aps["identb"][:, :])
    identf = consts.tile([128, 128], F32, tag="identf")
    nc.scalar.dma_start(out=identf[:], in_=aps["identf"][:, :])
    w3t = consts.tile([128, 128], BF16, tag="w3t")
    nc.scalar.dma_start(out=w3t[:], in_=w3t_d[:, :])
    w4t = []
    for q in range(2):
        t = consts.tile([128, C2], BF16, tag=f"w4t_{q}")
        nc.scalar.dma_start(out=t[:], in_=w4t_d[q * 128 : (q + 1) * 128, :])
        w4t.append(t)
    w5t = []
    for q in range(2):
        t = consts.tile([128, C], BF16, tag=f"w5t_{q}")
        nc.scalar.dma_start(out=t[:], in_=w5t_d[q * 128 : (q + 1) * 128, :])
        w5t.append(t)

    bias_t = {}
    if use_bias:
        b12row_d = aps["b12row"]
        b1_d, b3_d, b4_d, b5_d = aps["b1c"], aps["b3c"], aps["b4c"], aps["b5c"]
        brow1 = consts.tile([1, KM], F32, tag="brow1")
        nc.scalar.dma_start(out=brow1[:], in_=b12row_d[:, :])
        brow = consts.tile([128, KM], F32, tag="brow")
        nc.gpsimd.partition_broadcast(brow[:], brow1[:])
        bias_t["brow"] = brow
        b1 = consts.tile([128, 1], F32, tag="b1")
        nc.scalar.dma_start(out=b1[:], in_=b1_d[:, :])
        bias_t["b1"] = b1
        b3 = consts.tile([128, 1], F32, tag="b3")
        nc.scalar.dma_start(out=b3[:], in_=b3_d[:, :])
        bias_t["b3"] = b3
        b4 = []
        for q in range(2):
            t = consts.tile([128, 1], F32, tag=f"b4_{q}")
            nc.scalar.dma_start(out=t[:], in_=b4_d[q * 128 : (q + 1) * 128, :])
            b4.append(t)
        bias_t["b4"] = b4
        b5 = []
        for oc in range(CK):
            t = consts.tile([128, 1], F32, tag=f"b5_{oc}")
            nc.scalar.dma_start(out=t[:], in_=b5_d[oc * 128 : (oc + 1) * 128, :])
            b5.append(t)
        bias_t["b5"] = b5

    sm = ctx.enter_context(tc.tile_pool(name="sm", bufs=1))

    # ---- phase 1: x12T tiles + v accumulation ----
    with (
        tc.tile_pool(name="psA", bufs=3, space="PSUM") as psA,
        tc.tile_pool(name="vps", bufs=1, space="PSUM") as vps,
        tc.tile_pool(name="xtp", bufs=6) as xtp,
    ):
        v_ps = vps.tile([128, C2], F32, tag="v")

        # Warm the PE HAM clock-gate during the initial x-DMA window.
        wps = psA.tile([128, 128], F32, tag="warm", bufs=1)
        for _ in range(40):
            nc.tensor.matmul(wps[:], warm0[:], warm0[:], start=True, stop=True)

        def emit_v(s, xtile):
            nc.tensor.matmul(
                v_ps[:],
                xtile[:, 0:C4],
                xtile[:, C4:KM],
                start=(s == 0),
                stop=(s == NSUB - 1),
            )

        SKEW = 2
        pend = []
        for s in range(NSUB):
            goff = s * 128
            ps = psA.tile([128, KM], F32, tag="xts")
            for c in range(CK):
                nc.tensor.matmul(
                    ps[:],
                    xs(c, goff, 128),
                    w12[c][:],
                    start=(c == 0),
                    stop=(c == CK - 1),
                )
            xtile = xtp.tile([128, KM], BF16, tag="xt")
            if use_bias:
                nc.vector.tensor_tensor(
                    xtile[:], ps[:], bias_t["brow"][:], op=AL.add
                )
            else:
                if s % 2 == 0:
                    nc.scalar.copy(xtile[:], ps[:])
                else:
                    nc.vector.tensor_copy(xtile[:], ps[:])
            pend.append((s, xtile))
            if len(pend) > SKEW:
                emit_v(*pend.pop(0))
        while pend:
            emit_v(*pend.pop(0))

        # ---- softmax over all 32768 entries of v ----
        m1 = sm.tile([128, 1], F32, tag="m1")
        nc.vector.tensor_reduce(m1[:], v_ps[:], axis=AX.X, op=AL.max)
        mall = sm.tile([128, 1], F32, tag="mall")
        nc.gpsimd.partition_all_reduce(mall[:], m1[:], 128, bass_isa.ReduceOp.max)
        negm = sm.tile([128, 1], F32, tag="negm")
        nc.vector.tensor_scalar_mul(negm[:], mall[:], -1.0)
        e = sm.tile([128, C2], F32, tag="e")
        nc.scalar.activation(e[:], v_ps[:], AF.Exp, bias=negm[:], scale=1.0)

    s1 = sm.tile([128, 1], F32, tag="s1")
    nc.vector.tensor_reduce(s1[:], e[:], axis=AX.X, op=AL.add)
    sall = sm.tile([128, 1], F32, tag="sall")
    nc.gpsimd.partition_all_reduce(sall[:], s1[:], 128, bass_isa.ReduceOp.add)
    sinv = sm.tile([128, 1], F32, tag="sinv")
    nc.vector.reciprocal(sinv[:], sall[:])
    en = sm.tile([128, C2], BF16, tag="en")
    nc.vector.tensor_scalar_mul(en[:], e[:], sinv[:])

    # ---- small chain: conv3+relu, n^T, z, W^T ----
    wt = sm.tile([128, C], BF16, tag="wt")
    with tc.tile_pool(name="psB", bufs=2, space="PSUM") as psB:
        # keep-warm dummies: bridge the serial softmax window so the PE HAM
        # clock does not re-throttle before phase 2.  Shares the pT tag's
        # PSUM ring (the real transposes come later and just rotate it).
        wps2 = psB.tile([128, 128], F32, tag="pT", name="warm2")
        for _ in range(56):
            nc.tensor.matmul(wps2[:], warm0[:], warm0[:], start=True, stop=True)

        ps3 = psB.tile([128, C2], F32, tag="ps3")
        nc.tensor.matmul(ps3[:], w3t[:], en[:], start=True, stop=True)
        nsb = sm.tile([128, C2], F32, tag="nsb")
        # nsb = (e * sinv) + ps3 (+ b3)
        nc.vector.scalar_tensor_tensor(
            nsb[:], e[:], sinv[:], ps3[:], op0=AL.mult, op1=AL.add
        )
        if use_bias:
            nc.vector.tensor_scalar_add(nsb[:], nsb[:], bias_t["b3"][:])
        nc.vector.tensor_scalar_max(nsb[:], nsb[:], 0.0)

        nts = []
        for q in range(2):
            pT = psB.tile([128, 128], F32, tag="pT")
            nc.tensor.transpose(pT[:], nsb[:, q * 128 : (q + 1) * 128], identf[:])
            ntq = sm.tile([128, 128], BF16, tag=f"nt{q}")
            nc.scalar.copy(ntq[:], pT[:])
            nts.append(ntq)

        zs = []
        for mc in range(2):
            pz = psB.tile([128, 128], F32, tag="pz")
            for q in range(2):
                nc.tensor.matmul(
                    pz[:],
                    w4t[q][:, mc * 128 : (mc + 1) * 128],
                    nts[q][:],
                    start=(q == 0),
                    stop=(q == 1),
                )
            zq = sm.tile([128, 128], BF16, tag=f"z{mc}")
            if use_bias:
                nc.scalar.add(zq[:], pz[:], bias_t["b4"][mc][:])
            else:
                nc.scalar.copy(zq[:], pz[:])
            zs.append(zq)

        pW = psB.tile([128, C], F32, tag="pW")
        for mc in range(2):
            nc.tensor.matmul(
                pW[:], zs[mc][:], w5t[mc][:], start=(mc == 0), stop=(mc == 1)
            )
        nc.scalar.copy(wt[:], pW[:])

    # ---- phase 2: x1 recompute (bf16), x_res = W @ x1, residual, DMA out ----
    # Output staged in [128, 1536] bf16 tiles (one per (oc, group-of-3
    # hw-tiles)); all 24 out-DMAs issue on the sync HWDGE queue.  The
    # residual add cycles PE (identity matmul into the x_res PSUM bank,
    # then ACT copy to staging) and a DVE tensor_tensor that writes
    # staging directly, to balance the three engines under the DMA floor.
    with (
        tc.tile_pool(name="psC", bufs=2, space="PSUM") as psC,
        tc.tile_pool(name="psD", bufs=5, space="PSUM") as psD,
        tc.tile_pool(name="x1p", bufs=4) as x1p,
        tc.tile_pool(name="outp", bufs=8) as outp,
    ):
        stage = {}  # oc -> current [128,1536] staging tile

        def emit_tail(t, x1tile):
            goff_t = t * 512
            g, gi = divmod(t, 3)
            for oc in range(CK):
                if gi == 0:
                    stage[oc] = outp.tile(
                        [128, 1536], BF16, tag=f"st{oc}", name=f"st{oc}_{g}"
                    )
                st = stage[oc]
                dst = st[:, gi * 512 : (gi + 1) * 512]
                pr = psD.tile([128, 512], F32, tag="pr")
                use_pe = (t * CK + oc) % 4 == 0
                nc.tensor.matmul(
                    pr[:],
                    wt[:, oc * 128 : (oc + 1) * 128],
                    x1tile[:],
                    start=True,
                    stop=not use_pe,
                )
                b5s = bias_t["b5"][oc][:] if use_bias else None
                if use_pe:
                    nc.tensor.matmul(
                        pr[:],
                        identb[:],
                        xs(oc, goff_t, 512),
                        start=False,
                        stop=True,
                    )
                    if b5s is not None:
                        nc.scalar.add(dst, pr[:], b5s)
                    else:
                        nc.scalar.copy(dst, pr[:])
                else:
                    if b5s is not None:
                        nc.vector.scalar_tensor_tensor(
                            dst, pr[:], b5s, xs(oc, goff_t, 512),
                            op0=AL.add, op1=AL.add,
                        )
                    else:
                        nc.vector.tensor_tensor(
                            dst, pr[:], xs(oc, goff_t, 512), op=AL.add
                        )
                if gi == 2:
                    nc.sync.dma_start(
                        out=out_d[
                            oc * 128 : (oc + 1) * 128, g * 1536 : (g + 1) * 1536
                        ],
                        in_=st[:],
                    )

        prev = None
        for t in range(NT):
            goff_t = t * 512
            px1 = psC.tile([128, 512], F32, tag="px1")
            for c in range(CK):
                nc.tensor.matmul(
                    px1[:],
                    w12[c][:, 0:C4],
                    xs(c, goff_t, 512),
                    start=(c == 0),
                    stop=(c == CK - 1),
                )
            x1tile = x1p.tile([128, 512], BF16, tag="x1t")
            if use_bias:
                nc.scalar.add(x1tile[:], px1[:], bias_t["b1"][:])
            else:
                nc.scalar.copy(x1tile[:], px1[:])
            if prev is not None:
                emit_tail(*prev)
            prev = (t, x1tile)
        emit_tail(*prev)


def _build(use_bias):
    nc = bacc.Bacc("TRN2", target_bir_lowering=False, debug=False, num_devices=N_CORES)
    aps = {
        "x": nc.dram_tensor("x", [128, XCOLS], BF16, kind="ExternalInput").ap(),
        "w12t": nc.dram_tensor("w12t", [C, KM], BF16, kind="ExternalInput").ap(),
        "w3t": nc.dram_tensor("w3t", [C4, C4], BF16, kind="ExternalInput").ap(),
        "w4t": nc.dram_tensor("w4t", [C2, C2], BF16, kind="ExternalInput").ap(),
        "w5t": nc.dram_tensor("w5t", [C2, C], BF16, kind="ExternalInput").ap(),
        "identf": nc.dram_tensor("identf", [128, 128], F32, kind="ExternalInput").ap(),
        "identb": nc.dram_tensor("identb", [128, 128], BF16, kind="ExternalInput").ap(),
        "out": nc.dram_tensor("out", [C, HW], BF16, kind="ExternalOutput").ap(),
    }
    if use_bias:
        aps["b12row"] = nc.dram_tensor(
            "b12row", [1, KM], F32, kind="ExternalInput"
        ).ap()
        aps["b1c"] = nc.dram_tensor("b1c", [C4, 1], F32, kind="ExternalInput").ap()
        aps["b3c"] = nc.dram_tensor("b3c", [C4, 1], F32, kind="ExternalInput").ap()
        aps["b4c"] = nc.dram_tensor("b4c", [C2, 1], F32, kind="ExternalInput").ap()
        aps["b5c"] = nc.dram_tensor("b5c", [C, 1], F32, kind="ExternalInput").ap()

    from contextlib import ExitStack

    with tile.TileContext(nc) as tc:
        with ExitStack() as ctx:
            _emit(ctx, tc, aps, use_bias)
    nc.compile()
    return nc


_CACHE = {}


def _pack_x(xb_bf):
    """[512, 9216] bf16 -> packed [128, 36864]: block0 as 3 sub-blocks of
    (c x 512), blocks 1..5 as (c x 1536)."""
    xc = xb_bf.reshape(CK, 128, HW)
    parts = []
    for sub in range(3):
        parts.append(xc[:, :, sub * 512 : (sub + 1) * 512])  # [4,128,512]
    for b in range(1, NBLK):
        parts.append(xc[:, :, b * BLK : (b + 1) * BLK])      # [4,128,1536]
    return np.concatenate(
        [p.transpose(1, 0, 2).reshape(128, -1) for p in parts], axis=1
    )


def _run(inputs, trace=False, **run_kwargs):
    x = np.ascontiguousarray(np.asarray(inputs["x"], dtype=np.float32))
    assert x.shape == (N_CORES, C, H, W_IMG), x.shape
    w1 = np.asarray(inputs["w1"], dtype=np.float32)
    w2 = np.asarray(inputs["w2"], dtype=np.float32)
    w3 = np.asarray(inputs["w3"], dtype=np.float32)
    w4 = np.asarray(inputs["w4"], dtype=np.float32)
    w5 = np.asarray(inputs["w5"], dtype=np.float32)
    b1 = np.asarray(inputs["b1"], dtype=np.float32)
    b2 = np.asarray(inputs["b2"], dtype=np.float32)
    b3 = np.asarray(inputs["b3"], dtype=np.float32)
    b4 = np.asarray(inputs["b4"], dtype=np.float32)
    b5 = np.asarray(inputs["b5"], dtype=np.float32)
    use_bias = bool(
        np.any(b1) or np.any(b2) or np.any(b3) or np.any(b4) or np.any(b5)
    )

    if use_bias not in _CACHE:
        _CACHE[use_bias] = _build(use_bias)
    nc = _CACHE[use_bias]

    bf = ml_dtypes.bfloat16
    w12t = np.ascontiguousarray(
        np.concatenate([w1.T, w2.T], axis=1), dtype=bf
    )  # [512, 384]
    w3t = np.ascontiguousarray(w3.T, dtype=bf)
    w4t = np.ascontiguousarray(w4.T, dtype=bf)
    w5t = np.ascontiguousarray(w5.T, dtype=bf)

    shared = {
        "w12t": w12t,
        "w3t": w3t,
        "w4t": w4t,
        "w5t": w5t,
        "identf": np.eye(128, dtype=np.float32),
        "identb": np.eye(128, dtype=bf),
    }
    if use_bias:
        shared["b12row"] = np.ascontiguousarray(
            np.concatenate([b1, b2])[None, :], dtype=np.float32
        )
        shared["b1c"] = np.ascontiguousarray(b1[:, None])
        shared["b3c"] = np.ascontiguousarray(b3[:, None])
        shared["b4c"] = np.ascontiguousarray(b4[:, None])
        shared["b5c"] = np.ascontiguousarray(b5[:, None])

    in_maps = [
        {
            "x": np.ascontiguousarray(_pack_x(x[b].reshape(C, HW).astype(bf))),
            **shared,
        }
        for b in range(N_CORES)
    ]
    res = run_bass_kernel_spmd(
        nc, in_maps, core_ids=list(range(N_CORES)), trace=trace, **run_kwargs
    )
    out = np.stack(
        [
            np.asarray(res.results[b]["out"]).astype(np.float32).reshape(C, H, W_IMG)
            for b in range(N_CORES)
        ]
    )
    return out, res


def kernel(**inputs):
    out, _ = _run(inputs, trace=False)
    return out


# revision 11
# speedup vs baseline: 1.6198x; 1.2306x over previous
"""Trainium2 Bass kernel for the GCM (global context module) problem.

Computation per batch sample b (x_b = x[b] viewed as [C=512, HW=9216]):
    x1 = w1 @ x_b                      [128, HW]
    v  = softmax_all(x1 @ x2^T)        [128, 256]  (softmax over all 32768)
    n  = relu(v + w3 @ v)              [128, 256]
    z  = w4 @ n^T                      [256, 128]
    W  = w5 @ z                        [512, 128]  (collapses y/conv5: w5@(z@x1) == (w5@z)@x1)
    out = x_b + W @ x1                 [512, HW]

Sharding: data-parallel over batch, one sample per NeuronCore (8 cores).

v4 strategy: bf16 I/O + Gram-trick phase 1 with an fp8 x^T operand.
  - The x2 GEMM (2/3 of phase-1 FLOPs) is eliminated:
        v = x1 @ x2^T = (x1 @ x^T) @ w2^T = A @ w2^T
    A is accumulated on the PE from fp8 operands (x1T subtiles produced by
    on-chip PE transposes of the k-major x1, x^T uploaded host-transposed
    in fp8).  Phase-1 PE work drops from 1.81G+0.30G MACs (x1,x2 hw-major
    + v) to 0.60G (x1 k-major, long streams) + 0.15G (transposes) + 0.60G
    (A) + small.
  - Numerics: the softmax is a near-one-hot argmax (top-2 logit gaps
    6.7..102 across the batch) and |x_res| <= 0.04 vs |x| <= 5.1, so fp8
    noise in the logits is harmless; measured end-to-end rel err ~5e-3 vs
    the 2e-2 gate, dominated by the bf16 x/out passthrough rounding.
  - x uploaded bf16 host-repacked (one [128, 36864] SBUF tile, block-major
    with finer-grained block 0, 8 line-rate DMAs); x^T uploaded fp8
    host-transposed ([128, 36864], 6 DMAs) interleaved block-by-block.
  - out written bf16, staged [128,1536], all out-DMAs on the sync HWDGE
    queue; residual add balanced across PE / DVE / ACT.
  - PE warmup against a memset tile (no DMA dependency); dummy matmuls
    bridge the serial softmax window to keep the HAM clock-gate released.
"""

import numpy as np
import ml_dtypes

import concourse.bass as bass
import concourse.tile as tile
from concourse import bacc, mybir, bass_isa
from concourse.bass_utils import run_bass_kernel_spmd

F32 = mybir.dt.float32
BF16 = mybir.dt.bfloat16
FP8 = mybir.dt.float8e4
AX = mybir.AxisListType
AL = mybir.AluOpType
AF = mybir.ActivationFunctionType

N_CORES = 8
C = 512
H = W_IMG = 96
HW = H * W_IMG          # 9216
CK = C // 128           # 4 chunks of channels
NBLK = 6                # x blocks along hw
BLK = HW // NBLK        # 1536
NSUB = HW // 128        # 72 subtiles
NT = HW // 512          # 18 hw tiles of 512
C4 = C // 4             # 128
C2 = C // 2             # 256
KM = C4 + C2            # 384 = concat(w1T, w2T) free size
XCOLS = CK * HW         # free size of the packed x tile (36864)


def _xcol(c, goff):
    """Column in the packed x tile for channel-chunk c, global hw offset."""
    b, off = divmod(goff, BLK)
    if b == 0:
        sub, o = divmod(off, 512)
        return sub * (CK * 512) + c * 512 + o
    return b * (CK * BLK) + c * BLK + off


def _emit(ctx, tc, aps, use_bias):
    nc = tc.nc
    x_d = aps["x"]
    xt8_d = aps["xt8"]
    w12t_d = aps["w12t"]
    w3t_d = aps["w3t"]
    w4t_d = aps["w4t"]
    w5t_d = aps["w5t"]
    out_d = aps["out"]

    consts = ctx.enter_context(tc.tile_pool(name="consts", bufs=1))

    # Warmup operand: memset, so the first PE matmuls have no DMA dependency.
    warm0 = consts.tile([128, 128], BF16, tag="warm0")
    nc.vector.memset(warm0[:], 0)

    # ---- weights first on sync (small, gate phase-1 start), then x ----
    w12 = []
    for c in range(CK):
        t = consts.tile([128, KM], BF16, tag=f"w12_{c}")
        nc.sync.dma_start(out=t[:], in_=w12t_d[c * 128 : (c + 1) * 128, :])
        w12.append(t)

    # x resident: one packed [128, 36864] bf16 tile; x^T fp8 likewise.
    # Interleave xT blocks behind the matching x blocks (A-matmuls for
    # block b trail the x1-matmuls by two pipeline stages).
    xpool = ctx.enter_context(tc.tile_pool(name="x", bufs=1))
    xall = xpool.tile([128, XCOLS], BF16, tag="xall", name="xall")
    xt8 = xpool.tile([128, XCOLS], FP8, tag="xt8", name="xt8")
    x1sb = xpool.tile([128, HW], BF16, tag="x1sb", name="x1sb")
    WSUB = CK * 512
    for sub in range(3):
        nc.sync.dma_start(
            out=xall[:, sub * WSUB : (sub + 1) * WSUB],
            in_=x_d[:, sub * WSUB : (sub + 1) * WSUB],
        )
    WBLK = CK * BLK
    nc.sync.dma_start(out=xt8[:, 0:WBLK], in_=xt8_d[:, 0:WBLK])
    for b in range(1, NBLK):
        nc.sync.dma_start(
            out=xall[:, b * WBLK : (b + 1) * WBLK],
            in_=x_d[:, b * WBLK : (b + 1) * WBLK],
        )
        nc.sync.dma_start(
            out=xt8[:, b * WBLK : (b + 1) * WBLK],
            in_=xt8_d[:, b * WBLK : (b + 1) * WBLK],
        )

    def xs(c, goff, width):
        col = _xcol(c, goff)
        return xall[:, col : col + width]

    # ---- small constants on the scalar HWDGE queue (parallel, needed late)
    identb = consts.tile([128, 128], BF16, tag="identb")
    nc.scalar.dma_start(out=identb[:], in_=aps["identb"][:, :])
    identf = consts.tile([128, 128], F32, tag="identf")
    nc.scalar.dma_start(out=identf[:], in_=aps["identf"][:, :])
    w3t = consts.tile([128, 128], BF16, tag="w3t")
    nc.scalar.dma_start(out=w3t[:], in_=w3t_d[:, :])
    w4t = []
    for q in range(2):
        t = consts.tile([128, C2], BF16, tag=f"w4t_{q}")
        nc.scalar.dma_start(out=t[:], in_=w4t_d[q * 128 : (q + 1) * 128, :])
        w4t.append(t)
    w5t = []
    for q in range(2):
        t = consts.tile([128, C], BF16, tag=f"w5t_{q}")
        nc.scalar.dma_start(out=t[:], in_=w5t_d[q * 128 : (q + 1) * 128, :])
        w5t.append(t)

    bias_t = {}
    if use_bias:
        b12row_d = aps["b12row"]
        b1_d, b3_d, b4_d, b5_d = aps["b1c"], aps["b3c"], aps["b4c"], aps["b5c"]
        brow1 = consts.tile([1, KM], F32, tag="brow1")
        nc.scalar.dma_start(out=brow1[:], in_=b12row_d[:, :])
        brow = consts.tile([128, KM], F32, tag="brow")
        nc.gpsimd.partition_broadcast(brow[:], brow1[:])
        bias_t["brow"] = brow
        b1 = consts.tile([128, 1], F32, tag="b1")
        nc.scalar.dma_start(out=b1[:], in_=b1_d[:, :])
        bias_t["b1"] = b1
        b3 = consts.tile([128, 1], F32, tag="b3")
        nc.scalar.dma_start(out=b3[:], in_=b3_d[:, :])
        bias_t["b3"] = b3
        b4 = []
        for q in range(2):
            t = consts.tile([128, 1], F32, tag=f"b4_{q}")
            nc.scalar.dma_start(out=t[:], in_=b4_d[q * 128 : (q + 1) * 128, :])
            b4.append(t)
        bias_t["b4"] = b4
        b5 = []
        for oc in range(CK):
            t = consts.tile([128, 1], F32, tag=f"b5_{oc}")
            nc.scalar.dma_start(out=t[:], in_=b5_d[oc * 128 : (oc + 1) * 128, :])
            b5.append(t)
        bias_t["b5"] = b5

    sm = ctx.enter_context(tc.tile_pool(name="sm", bufs=1))

    # ---- phase 1: x1 k-major, PE transposes, A = x1 @ x^T, v = A @ w2^T ----
    with (
        tc.tile_pool(name="psA", bufs=2, space="PSUM") as psA,
        tc.tile_pool(name="psT", bufs=2, space="PSUM") as psT,
        tc.tile_pool(name="apsP", bufs=1, space="PSUM") as apsP,
        tc.tile_pool(name="vps", bufs=1, space="PSUM") as vps,
        tc.tile_pool(name="x18p", bufs=3) as x18p,
    ):
        A_ps = apsP.tile([128, C], F32, tag="A")
        v_ps = vps.tile([128, C2], F32, tag="v")

        # Warm the PE HAM clock-gate during the initial DMA window.
        wps = psA.tile([128, 128], F32, tag="warm", bufs=1)
        for _ in range(40):
            nc.tensor.matmul(wps[:], warm0[:], warm0[:], start=True, stop=True)

        def x1_group(t):
            px1 = psA.tile([128, 512], F32, tag="px1")
            for c in range(CK):
                nc.tensor.matmul(
                    px1[:],
                    w12[c][:, 0:C4],
                    xs(c, t * 512, 512),
                    start=(c == 0),
                    stop=(c == CK - 1),
                )
            dstx1 = x1sb[:, t * 512 : (t + 1) * 512]
            if use_bias:
                nc.scalar.add(dstx1, px1[:], bias_t["b1"][:])
            elif t % 2 == 0:
                nc.scalar.copy(dstx1, px1[:])
            else:
                nc.vector.tensor_copy(dstx1, px1[:])

        def transp_group(t):
            pT = psT.tile([128, 512], BF16, tag="pT")
            for j in range(4):
                nc.tensor.transpose(
                    pT[:, j * 128 : (j + 1) * 128],
                    x1sb[:, t * 512 + j * 128 : t * 512 + (j + 1) * 128],
                    identb[:],
                )
            x18 = x18p.tile([128, 512], FP8, tag="x18")
            nc.vector.tensor_copy(x18[:], pT[:])
            return x18

        def a_group(t, x18):
            for j in range(4):
                s = t * 4 + j
                nc.tensor.matmul(
                    A_ps[:],
                    x18[:, j * 128 : (j + 1) * 128],
                    xt8[:, s * 512 : (s + 1) * 512],
                    start=(s == 0),
                    stop=(s == NSUB - 1),
                )

        pend = []
        for t in range(NT):
            x1_group(t)
            if t >= 1:
                pend.append((t - 1, transp_group(t - 1)))
            if t >= 2:
                a_group(*pend.pop(0))
        pend.append((NT - 1, transp_group(NT - 1)))
        while pend:
            a_group(*pend.pop(0))

        # ---- v = A @ w2^T (transpose A on the PE first) ----
        asb = sm.tile([128, C], BF16, tag="asb")
        nc.scalar.copy(asb[:], A_ps[:])
        pTv = psT.tile([128, 512], BF16, tag="pT")
        for q in range(CK):
            nc.tensor.transpose(
                pTv[:, q * 128 : (q + 1) * 128],
                asb[:, q * 128 : (q + 1) * 128],
                identb[:],
            )
        atp = sm.tile([128, C], BF16, tag="atp")
        nc.vector.tensor_copy(atp[:], pTv[:])
        for q in range(CK):
            nc.tensor.matmul(
                v_ps[:],
                atp[:, q * 128 : (q + 1) * 128],
                w12[q][:, C4:KM],
                start=(q == 0),
                stop=(q == CK - 1),
            )

        # ---- softmax over all 32768 entries of v ----
        m1 = sm.tile([128, 1], F32, tag="m1")
        nc.vector.tensor_reduce(m1[:], v_ps[:], axis=AX.X, op=AL.max)
        mall = sm.tile([128, 1], F32, tag="mall")
        nc.gpsimd.partition_all_reduce(mall[:], m1[:], 128, bass_isa.ReduceOp.max)
        negm = sm.tile([128, 1], F32, tag="negm")
        nc.vector.tensor_scalar_mul(negm[:], mall[:], -1.0)
        e = sm.tile([128, C2], F32, tag="e")
        nc.scalar.activation(e[:], v_ps[:], AF.Exp, bias=negm[:], scale=1.0)

    s1 = sm.tile([128, 1], F32, tag="s1")
    nc.vector.tensor_reduce(s1[:], e[:], axis=AX.X, op=AL.add)
    sall = sm.tile([128, 1], F32, tag="sall")
    nc.gpsimd.partition_all_reduce(sall[:], s1[:], 128, bass_isa.ReduceOp.add)
    sinv = sm.tile([128, 1], F32, tag="sinv")
    nc.vector.reciprocal(sinv[:], sall[:])
    en = sm.tile([128, C2], BF16, tag="en")
    nc.vector.tensor_scalar_mul(en[:], e[:], sinv[:])

    # ---- small chain: conv3+relu, n^T, z, W^T ----
    wt = sm.tile([128, C], BF16, tag="wt")
    with tc.tile_pool(name="psB", bufs=2, space="PSUM") as psB:
        # keep-warm dummies bridge the serial softmax window (shares the pT
        # tag's PSUM ring; the real transposes just rotate it later).
        wps2 = psB.tile([128, 128], F32, tag="pT", name="warm2")
        for _ in range(56):
            nc.tensor.matmul(wps2[:], warm0[:], warm0[:], start=True, stop=True)

        ps3 = psB.tile([128, C2], F32, tag="ps3")
        nc.tensor.matmul(ps3[:], w3t[:], en[:], start=True, stop=True)
        nsb = sm.tile([128, C2], F32, tag="nsb")
        # nsb = (e * sinv) + ps3 (+ b3)
        nc.vector.scalar_tensor_tensor(
            nsb[:], e[:], sinv[:], ps3[:], op0=AL.mult, op1=AL.add
        )
        if use_bias:
            nc.vector.tensor_scalar_add(nsb[:], nsb[:], bias_t["b3"][:])
        nc.vector.tensor_scalar_max(nsb[:], nsb[:], 0.0)

        nts = []
        for q in range(2):
            pT = psB.tile([128, 128], F32, tag="pT")
            nc.tensor.transpose(pT[:], nsb[:, q * 128 : (q + 1) * 128], identf[:])
            ntq = sm.tile([128, 128], BF16, tag=f"nt{q}")
            nc.scalar.copy(ntq[:], pT[:])
            nts.append(ntq)

        zs = []
        for mc in range(2):
            pz = psB.tile([128, 128], F32, tag="pz")
            for q in range(2):
                nc.tensor.matmul(
                    pz[:],
                    w4t[q][:, mc * 128 : (mc + 1) * 128],
                    nts[q][:],
                    start=(q == 0),
                    stop=(q == 1),
                )
            zq = sm.tile([128, 128], BF16, tag=f"z{mc}")
            if use_bias:
                nc.scalar.add(zq[:], pz[:], bias_t["b4"][mc][:])
            else:
                nc.scalar.copy(zq[:], pz[:])
            zs.append(zq)

        pW = psB.tile([128, C], F32, tag="pW")
        for mc in range(2):
            nc.tensor.matmul(
                pW[:], zs[mc][:], w5t[mc][:], start=(mc == 0), stop=(mc == 1)
            )
        nc.scalar.copy(wt[:], pW[:])

    # ---- phase 2: x_res = W @ x1 (x1 already resident), residual, out ----
    # Output staged in [128, 1536] bf16 tiles; all 24 out-DMAs on sync.
    # Residual alternates PE (identity matmul into the x_res PSUM bank +
    # ACT copy to staging) and DVE (tensor_tensor writes staging directly).
    with (
        tc.tile_pool(name="psD", bufs=6, space="PSUM") as psD,
        tc.tile_pool(name="outp", bufs=2) as outp,
    ):
        stage = {}

        for t in range(NT):
            g, gi = divmod(t, 3)
            x1t = x1sb[:, t * 512 : (t + 1) * 512]
            for oc in range(CK):
                if gi == 0:
                    stage[oc] = outp.tile(
                        [128, 1536], BF16, tag=f"st{oc}", name=f"st{oc}_{g}"
                    )
                st = stage[oc]
                dst = st[:, gi * 512 : (gi + 1) * 512]
                pr = psD.tile([128, 512], F32, tag="pr")
                use_pe = (t * CK + oc) % 2 == 0
                nc.tensor.matmul(
                    pr[:],
                    wt[:, oc * 128 : (oc + 1) * 128],
                    x1t,
                    start=True,
                    stop=not use_pe,
                )
                b5s = bias_t["b5"][oc][:] if use_bias else None
                if use_pe:
                    nc.tensor.matmul(
                        pr[:],
                        identb[:],
                        xs(oc, t * 512, 512),
                        start=False,
                        stop=True,
                    )
                    if b5s is not None:
                        nc.scalar.add(dst, pr[:], b5s)
                    else:
                        nc.scalar.copy(dst, pr[:])
                else:
                    if b5s is not None:
                        nc.vector.scalar_tensor_tensor(
                            dst, pr[:], b5s, xs(oc, t * 512, 512),
                            op0=AL.add, op1=AL.add,
                        )
                    else:
                        nc.vector.tensor_tensor(
                            dst, pr[:], xs(oc, t * 512, 512), op=AL.add
                        )
                if gi == 2:
                    nc.sync.dma_start(
                        out=out_d[
                            oc * 128 : (oc + 1) * 128, g * 1536 : (g + 1) * 1536
                        ],
                        in_=st[:],
                    )


def _build(use_bias):
    nc = bacc.Bacc("TRN2", target_bir_lowering=False, debug=False, num_devices=N_CORES)
    aps = {
        "x": nc.dram_tensor("x", [128, XCOLS], BF16, kind="ExternalInput").ap(),
        "xt8": nc.dram_tensor("xt8", [128, XCOLS], FP8, kind="ExternalInput").ap(),
        "w12t": nc.dram_tensor("w12t", [C, KM], BF16, kind="ExternalInput").ap(),
        "w3t": nc.dram_tensor("w3t", [C4, C4], BF16, kind="ExternalInput").ap(),
        "w4t": nc.dram_tensor("w4t", [C2, C2], BF16, kind="ExternalInput").ap(),
        "w5t": nc.dram_tensor("w5t", [C2, C], BF16, kind="ExternalInput").ap(),
        "identf": nc.dram_tensor("identf", [128, 128], F32, kind="ExternalInput").ap(),
        "identb": nc.dram_tensor("identb", [128, 128], BF16, kind="ExternalInput").ap(),
        "out": nc.dram_tensor("out", [C, HW], BF16, kind="ExternalOutput").ap(),
    }
    if use_bias:
        aps["b12row"] = nc.dram_tensor(
            "b12row", [1, KM], F32, kind="ExternalInput"
        ).ap()
        aps["b1c"] = nc.dram_tensor("b1c", [C4, 1], F32, kind="ExternalInput").ap()
        aps["b3c"] = nc.dram_tensor("b3c", [C4, 1], F32, kind="ExternalInput").ap()
        aps["b4c"] = nc.dram_tensor("b4c", [C2, 1], F32, kind="ExternalInput").ap()
        aps["b5c"] = nc.dram_tensor("b5c", [C, 1], F32, kind="ExternalInput").ap()

    from contextlib import ExitStack

    with tile.TileContext(nc) as tc:
        with ExitStack() as ctx:
            _emit(ctx, tc, aps, use_bias)
    nc.compile()
    return nc


_CACHE = {}


def _pack_x(xb_bf):
    """[512, 9216] bf16 -> packed [128, 36864]: block0 as 3 sub-blocks of
    (c x 512), blocks 1..5 as (c x 1536)."""
    xc = xb_bf.reshape(CK, 128, HW)
    parts = []
    for sub in range(3):
        parts.append(xc[:, :, sub * 512 : (sub + 1) * 512])
    for b in range(1, NBLK):
        parts.append(xc[:, :, b * BLK : (b + 1) * BLK])
    return np.concatenate(
        [p.transpose(1, 0, 2).reshape(128, -1) for p in parts], axis=1
    )


def _pack_xt8(xb):
    """[512, 9216] f32 -> fp8 x^T packed [128, 36864]:
    col s*512 + cc holds x[cc, s*128 + p] for partition p."""
    xt = xb.reshape(C, NSUB, 128).transpose(2, 1, 0)  # [128, 72, 512]
    return np.ascontiguousarray(
        xt.reshape(128, XCOLS).astype(ml_dtypes.float8_e4m3)
    )


def _run(inputs, trace=False, **run_kwargs):
    x = np.ascontiguousarray(np.asarray(inputs["x"], dtype=np.float32))
    assert x.shape == (N_CORES, C, H, W_IMG), x.shape
    w1 = np.asarray(inputs["w1"], dtype=np.float32)
    w2 = np.asarray(inputs["w2"], dtype=np.float32)
    w3 = np.asarray(inputs["w3"], dtype=np.float32)
    w4 = np.asarray(inputs["w4"], dtype=np.float32)
    w5 = np.asarray(inputs["w5"], dtype=np.float32)
    b1 = np.asarray(inputs["b1"], dtype=np.float32)
    b2 = np.asarray(inputs["b2"], dtype=np.float32)
    b3 = np.asarray(inputs["b3"], dtype=np.float32)
    b4 = np.asarray(inputs["b4"], dtype=np.float32)
    b5 = np.asarray(inputs["b5"], dtype=np.float32)
    use_bias = bool(
        np.any(b1) or np.any(b2) or np.any(b3) or np.any(b4) or np.any(b5)
    )

    if use_bias not in _CACHE:
        _CACHE[use_bias] = _build(use_bias)
    nc = _CACHE[use_bias]

    bf = ml_dtypes.bfloat16
    w12t = np.ascontiguousarray(
        np.concatenate([w1.T, w2.T], axis=1), dtype=bf
    )  # [512, 384]
    w3t = np.ascontiguousarray(w3.T, dtype=bf)
    w4t = np.ascontiguousarray(w4.T, dtype=bf)
    w5t = np.ascontiguousarray(w5.T, dtype=bf)

    shared = {
        "w12t": w12t,
        "w3t": w3t,
        "w4t": w4t,
        "w5t": w5t,
        "identf": np.eye(128, dtype=np.float32),
        "identb": np.eye(128, dtype=bf),
    }
    if use_bias:
        shared["b12row"] = np.ascontiguousarray(
            np.concatenate([b1, b2])[None, :], dtype=np.float32
        )
        shared["b1c"] = np.ascontiguousarray(b1[:, None])
        shared["b3c"] = np.ascontiguousarray(b3[:, None])
        shared["b4c"] = np.ascontiguousarray(b4[:, None])
        shared["b5c"] = np.ascontiguousarray(b5[:, None])

    in_maps = []
    for b in range(N_CORES):
        xb = x[b].reshape(C, HW)
        in_maps.append(
            {
                "x": np.ascontiguousarray(_pack_x(xb.astype(bf))),
                "xt8": _pack_xt8(xb.astype(bf).astype(np.float32)),
                **shared,
            }
        )
    res = run_bass_kernel_spmd(
        nc, in_maps, core_ids=list(range(N_CORES)), trace=trace, **run_kwargs
    )
    out = np.stack(
        [
            np.asarray(res.results[b]["out"]).astype(np.float32).reshape(C, H, W_IMG)
            for b in range(N_CORES)
        ]
    )
    return out, res


def kernel(**inputs):
    out, _ = _run(inputs, trace=False)
    return out
